# revision 25
# baseline (speedup 1.0000x reference)
"""Trainium2 Bass kernel for AtomToNucleotideTransform (segment softmax-pool + MLPs).

Sharding: 8 cores, each owns a contiguous range of 2500 nucleotides (55000 atoms).
Per-core padded to 2560 segments / 56320 atoms (= 440 tiles of 128 atoms,
20 seg-tiles of 128 segments, supertile = 1408 atoms = 64 segs, lcm(22,128)).

Pipeline per core (all weights replicated):
  atom phase (per seg-tile = 2816 atoms = 22 atom-tiles):
    - DMA emb [128,128] f32 tiles; cast bf16 on GpSimd; PE-transpose -> embT
    - mm1: t = W1e.T @ embT (bf16) + W1p.T @ physT (f32r)  [H, atoms] psum
    - tanh (ACT, bias=b1) -> bf16
    - s: per-tile matmul lhsT=tanh-slice, rhs=W2 -> s_cols psum [128 atoms, 1]
    - segment softmax: s_cols -> (PE transpose + 2 coarse DMAs) -> [seg, 22]
      layout; max/exp/sum/recip on DVE+ACT; w back to per-atom columns
    - wemb = w * emb (DVE, bf16 out); pooled += wemb.T @ S_window (PE, psum
      accumulate over 512-seg groups)
  nuc phase: comb matmul (f32r, sugar/phos weights folded on host), LayerNorm
  (natural layout after PE transpose), SiLU, rot/tr heads, quaternion norm.
"""

import os
import sys
from contextlib import ExitStack

import numpy as np

sys.path.insert(0, "/opt/trn_rl_repo")

import ml_dtypes  # noqa: E402
import concourse.bass as bass  # noqa: E402
import concourse.tile as tile  # noqa: E402
from concourse import mybir  # noqa: E402
from concourse.bass_utils import run_bass_kernel_spmd  # noqa: E402

F32 = mybir.dt.float32
F32R = mybir.dt.float32r
BF16 = mybir.dt.bfloat16
AF = mybir.ActivationFunctionType
AX = mybir.AxisListType

H = 128
APN = 22
NCORES = 8
N_NUC = 20000
NUC_C = N_NUC // NCORES            # 2500 real nucleotides per core
SUP_SEG = 64                       # segs per supertile (lcm(22,128)/22)
SUP_TILES = 11                     # atom tiles per supertile
ST_SEG = 128                       # segs per seg-tile
ST_TILES = 22                      # atom tiles per seg-tile
GRP_SEG = 512                      # segs per psum accumulation group
BF16_NP = ml_dtypes.bfloat16

# supertile-periodic indicator pattern
_OFFS, _WIDS, _SCOL = [], [], [0]
for _j in range(SUP_TILES):
    _lo = (128 * _j) // APN
    _hi = (128 * _j + 127) // APN
    _OFFS.append(_lo)
    _WIDS.append(_hi - _lo + 1)
    _SCOL.append(_SCOL[-1] + _WIDS[-1])
SW = _SCOL[-1]


def _indicator_np():
    s = np.zeros((128, SW), dtype=np.float32)
    for j in range(SUP_TILES):
        for p in range(128):
            seg = (128 * j + p) // APN
            s[p, _SCOL[j] + seg - _OFFS[j]] = 1.0
    return s.astype(BF16_NP)


def build_program(n_st: int, nuc_real: int):
    """Emit the bass program for one core handling n_st seg-tiles
    (n_st*128 padded segments) with nuc_real real nucleotides."""
    nuc_pad = n_st * ST_SEG
    a_pad = nuc_pad * APN
    nt = a_pad // 128
    grp_seg = min(GRP_SEG, nuc_pad)
    stpg = grp_seg // ST_SEG          # seg-tiles per psum group
    n_grp = nuc_pad // grp_seg
    assert nuc_pad % grp_seg == 0 and nt == n_st * ST_TILES

    nc = bass.Bass()
    dram = {}

    def din(name, shape, dt=F32):
        dram[name] = nc.dram_tensor(name, list(shape), dt, kind="ExternalInput")
        return dram[name]

    din("emb", (a_pad, 128), BF16)
    din("physT", (10, a_pad), BF16)
    din("sugxT", (8, nuc_pad))
    din("phoxT", (8, nuc_pad))
    din("W1e", (128, 128), BF16)
    din("W1p", (10, 128), BF16)
    din("W2", (128, 1), BF16)
    din("b1", (128, 1))
    din("S", (128, SW), BF16)
    din("zrow", (1, 512), BF16)
    din("identb", (128, 128), BF16)
    din("identf", (128, 128))
    din("C1", (128, 128))
    din("Wsug", (8, 128))
    din("Wpho", (8, 128))
    din("bc", (128, 1))
    din("g_rep", (128, 128))
    din("b_rep", (128, 128))
    din("eps_ln", (128, 1))
    din("rw1", (128, 128), BF16)
    din("rb1", (128, 1))
    din("rw2", (128, 4), BF16)
    din("rb2", (4, 1))
    din("tw1", (128, 128), BF16)
    din("tb1", (128, 1))
    din("tw2", (128, 3), BF16)
    din("tb2", (3, 1))

    o_quat = nc.dram_tensor("quat", [nuc_real, 4], F32, kind="ExternalOutput")
    o_trans = nc.dram_tensor("trans", [nuc_real, 3], F32, kind="ExternalOutput")
    o_nuc = nc.dram_tensor("nuc", [nuc_real, 128], F32, kind="ExternalOutput")

    with tile.TileContext(nc) as tc, ExitStack() as ctx:
        cpool = ctx.enter_context(tc.tile_pool(name="consts", bufs=1))
        xpool = ctx.enter_context(tc.tile_pool(name="xf", bufs=4))
        etpool = ctx.enter_context(tc.tile_pool(name="embT", bufs=4))
        thpool = ctx.enter_context(tc.tile_pool(name="tanh", bufs=8))
        ptpool = ctx.enter_context(tc.tile_pool(name="physT", bufs=2))
        smpool = ctx.enter_context(tc.tile_pool(name="smax", bufs=5))
        wepool = ctx.enter_context(tc.tile_pool(name="wemb", bufs=4))
        nupool = ctx.enter_context(tc.tile_pool(name="nuc", bufs=4))
        bigpool = ctx.enter_context(tc.tile_pool(name="big", bufs=1))
        flpool = ctx.enter_context(tc.tile_pool(name="flat", bufs=3))
        ps_t = ctx.enter_context(tc.tile_pool(name="ps_t", bufs=2, space="PSUM"))
        ps_e = ctx.enter_context(tc.tile_pool(name="ps_e", bufs=1, space="PSUM"))
        ps_s = ctx.enter_context(tc.tile_pool(name="ps_s", bufs=3, space="PSUM"))
        ps_p = ctx.enter_context(tc.tile_pool(name="ps_p", bufs=2, space="PSUM"))

        # ---- load constants into SBUF
        cst = {}
        for name, dt in [
            ("W1e", BF16), ("W1p", BF16), ("W2", BF16), ("b1", F32),
            ("S", BF16), ("zrow", BF16), ("identb", BF16), ("identf", F32),
            ("C1", F32), ("Wsug", F32), ("Wpho", F32), ("bc", F32),
            ("g_rep", F32), ("b_rep", F32), ("eps_ln", F32),
            ("rw1", BF16), ("rb1", F32), ("rw2", BF16), ("rb2", F32),
            ("tw1", BF16), ("tb1", F32), ("tw2", BF16), ("tb2", F32),
        ]:
            t = cpool.tile(list(dram[name].shape), dt, tag=name)
            nc.sync.dma_start(t[:], dram[name][:])
            cst[name] = t

        pooled_sb = bigpool.tile([128, nuc_pad], F32, tag="pooled")
        nucT_sb = bigpool.tile([128, nuc_pad], BF16, tag="nucT")

        embv = dram["emb"]
        phv = dram["physT"]

        # batches of atom tiles within a seg-tile for the N-dim of mm1
        BATCHES = [(0, 4), (4, 4), (8, 4), (12, 4), (16, 4), (20, 2)]

        pooled_holder = [None]
        stash = {}

        # batches of atom tiles within a seg-tile for the N-dim of mm1
        def stage_a(st):
            """DMA + transpose + mm1 + tanh + attention logits for seg-tile st."""
            a0 = st * ST_TILES * 128
            xfst = xpool.tile([128, ST_TILES * 128], BF16, tag="xf")
            nc.sync.dma_start(
                xfst[:].rearrange("p (k f) -> p k f", f=128),
                embv[a0:a0 + ST_TILES * 128, :].rearrange(
                    "(k p) f -> p k f", p=128))
            ptst = ptpool.tile([10, ST_TILES * 128], BF16, tag="pt")
            nc.sync.dma_start(ptst[:], phv[:, a0:a0 + ST_TILES * 128])
            xf = [xfst[:, k * 128:(k + 1) * 128] for k in range(ST_TILES)]
            tanh_tiles = []
            for bi, (k0, knum) in enumerate(BATCHES):
                nb = knum * 128
                embT = etpool.tile([128, 512], BF16, tag="embT")
                nc.sync.dma_start(
                    embT[:, 0:nb],
                    embv[a0 + k0 * 128:a0 + k0 * 128 + nb, :],
                    transpose=True)
                tps = ps_t.tile([128, 512], F32, tag="tps")
                nc.tensor.matmul(tps[:, 0:nb], cst["W1e"][:], embT[:, 0:nb],
                                 start=True, stop=False, skip_group_check=True)
                nc.tensor.matmul(tps[:, 0:nb], cst["W1p"][:],
                                 ptst[:, k0 * 128:k0 * 128 + nb],
                                 start=False, stop=True, skip_group_check=True)
                th = thpool.tile([128, 512], BF16, tag="tanh")
                nc.scalar.activation(th[:, 0:nb], tps[:, 0:nb], AF.Tanh,
                                     bias=cst["b1"][:, 0:1])
                tanh_tiles.append((th, nb))
            scps = ps_s.tile([128, 160], F32, tag="sps")
            for k in range(ST_TILES):
                th, _ = tanh_tiles[k // 4]
                nc.tensor.matmul(
                    scps[:, k:k + 1], th[:, (k % 4) * 128:(k % 4 + 1) * 128],
                    cst["W2"][:], start=True, stop=True, skip_group_check=True)
            scols = smpool.tile([128, ST_TILES], F32, tag="scols")
            nc.vector.tensor_copy(scols[:], scps[:, 0:ST_TILES])
            stash[st] = (xf, scols)

        def stage_b(st):
            """Segment softmax + weighted pooling for seg-tile st."""
            xf, scols = stash.pop(st)
            if st % stpg == 0:
                pooled_holder[0] = ps_p.tile([128, grp_seg], F32,
                                             name="pooled_ps", tag="pooled_ps")
                nc.tensor.matmul(
                    pooled_holder[0][:, :], cst["zrow"][0:1, 0:128],
                    cst["zrow"][0:1, 0:grp_seg], start=True, stop=False,
                    skip_group_check=True)
            pooled_cur = pooled_holder[0]

            swps = ps_s.tile([128, 160], F32, tag="sps")
            nc.tensor.transpose(swps[0:ST_TILES, 0:128], scols[:],
                                cst["identf"][:])
            stsb = smpool.tile([ST_TILES, 128], F32, tag="stsb")
            nc.vector.tensor_copy(stsb[:], swps[0:ST_TILES, 0:128])
            sflat = flpool.tile([1, ST_TILES * 128], F32, tag="flat")
            nc.scalar.dma_start(
                sflat[0:1, :].rearrange("o (c p) -> o c p", p=128), stsb[:])
            sseg = smpool.tile([ST_SEG, APN], F32, tag="sseg")
            nc.scalar.dma_start(
                sseg[:], sflat[0:1, :].rearrange("o (n k) -> o n k", k=APN))

            mrow = smpool.tile([ST_SEG, 1], F32, tag="mrow")
            nc.vector.reduce_max(mrow[:], sseg[:], axis=AX.X)
            nm = smpool.tile([ST_SEG, 1], F32, tag="nm")
            nc.vector.tensor_scalar_mul(nm[:], mrow[:], -1.0)
            eseg = smpool.tile([ST_SEG, APN], F32, tag="eseg")
            den = smpool.tile([ST_SEG, 1], F32, tag="den")
            nc.scalar.activation(eseg[:], sseg[:], AF.Exp, bias=nm[:, 0:1],
                                 accum_out=den[:, 0:1])
            rden = smpool.tile([ST_SEG, 1], F32, tag="rden")
            nc.vector.reciprocal(rden[:], den[:])
            wseg = smpool.tile([ST_SEG, APN], F32, tag="wseg")
            nc.vector.tensor_scalar_mul(wseg[:], eseg[:], rden[:, 0:1])

            wflat = flpool.tile([1, ST_TILES * 128], F32, tag="flat")
            nc.scalar.dma_start(
                wflat[0:1, :].rearrange("o (n k) -> o n k", k=APN), wseg[:])
            wtsb = smpool.tile([ST_TILES, 128], F32, tag="wtsb")
            nc.scalar.dma_start(
                wtsb[:], wflat[0:1, :].rearrange("o (c p) -> o c p", p=128))
            nc.tensor.transpose(swps[:, 128:128 + ST_TILES], wtsb[:],
                                cst["identf"][0:ST_TILES, 0:ST_TILES])
            wcols = smpool.tile([128, ST_TILES], F32, tag="wcols")
            nc.vector.tensor_copy(wcols[:], swps[:, 128:128 + ST_TILES])

            for k in range(ST_TILES):
                jg = st * ST_TILES + k
                jj = jg % SUP_TILES
                seg0 = (jg // SUP_TILES) * SUP_SEG + _OFFS[jj]
                wdt = _WIDS[jj]
                col = seg0 - (st // stpg) * grp_seg
                we = wepool.tile([128, 128], BF16, tag="wemb")
                nc.vector.tensor_scalar_mul(we[:], xf[k], wcols[:, k:k + 1])
                nc.tensor.matmul(
                    pooled_cur[:, col:col + wdt], we[:],
                    cst["S"][:, _SCOL[jj]:_SCOL[jj] + wdt],
                    start=False,
                    stop=(st % stpg == stpg - 1 and k == ST_TILES - 1),
                    skip_group_check=True)

            if st % stpg == stpg - 1:
                g = st // stpg
                nc.vector.tensor_copy(
                    pooled_sb[:, g * grp_seg:(g + 1) * grp_seg],
                    pooled_cur[:])

        # ================= atom phase (software-pipelined) ================
        lag = 2 if n_st > 2 else 1
        for st in range(n_st):
            stage_a(st)
            if st >= lag:
                stage_b(st - lag)
        for st in range(n_st - lag, n_st):
            stage_b(st)

        # ================= nucleotide phase =================
        for g in range(n_grp):
            c0 = g * grp_seg
            sgx = ptpool.tile([8, grp_seg], F32, tag="sgx")
            nc.sync.dma_start(sgx[:], dram["sugxT"][:, c0:c0 + grp_seg])
            pgx = ptpool.tile([8, grp_seg], F32, tag="pgx")
            nc.sync.dma_start(pgx[:], dram["phoxT"][:, c0:c0 + grp_seg])
            zps = ps_t.tile([128, grp_seg], F32, tag="tps")
            nc.tensor.matmul(zps[:], cst["C1"][:],
                             pooled_sb[:, c0:c0 + grp_seg],
                             start=True, stop=False, skip_group_check=True)
            nc.tensor.matmul(zps[:], cst["Wsug"][:], sgx[:],
                             start=False, stop=False, skip_group_check=True)
            nc.tensor.matmul(zps[:], cst["Wpho"][:], pgx[:],
                             start=False, stop=True, skip_group_check=True)
            zsb = nupool.tile([128, grp_seg], F32, tag="zsb")
            nc.vector.tensor_scalar_add(zsb[:], zps[:], cst["bc"][:, 0:1])

            for t in range(grp_seg // 128):
                gt = g * stpg + t
                row0 = gt * 128
                ztp = ps_e.tile([128, 128], F32, tag="teps")
                nc.tensor.transpose(ztp[:], zsb[:, t * 128:(t + 1) * 128],
                                    cst["identf"][:])
                musum = nupool.tile([128, 1], F32, tag="musum")
                nc.vector.reduce_sum(musum[:], ztp[:], axis=AX.X)
                mu = nupool.tile([128, 1], F32, tag="mu")
                nc.vector.tensor_scalar_mul(mu[:], musum[:], 1.0 / 128.0)
                zc = nupool.tile([128, 128], F32, tag="zc")
                nc.vector.tensor_scalar_sub(zc[:], ztp[:], mu[:, 0:1])
                sq = nupool.tile([128, 128], F32, tag="sq")
                nc.vector.tensor_tensor(sq[:], zc[:], zc[:],
                                        op=mybir.AluOpType.mult)
                ssum = nupool.tile([128, 1], F32, tag="ssum")
                nc.vector.reduce_sum(ssum[:], sq[:], axis=AX.X)
                std = nupool.tile([128, 1], F32, tag="std")
                nc.scalar.activation(std[:], ssum[:], AF.Sqrt,
                                     bias=cst["eps_ln"][:, 0:1],
                                     scale=1.0 / 128.0)
                rstd = nupool.tile([128, 1], F32, tag="rstd")
                nc.vector.reciprocal(rstd[:], std[:])
                zn1 = nupool.tile([128, 128], F32, tag="zn1")
                nc.vector.tensor_scalar_mul(zn1[:], zc[:], rstd[:, 0:1])
                zn2 = nupool.tile([128, 128], F32, tag="zn2")
                nc.vector.tensor_tensor(zn2[:], zn1[:], cst["g_rep"][:],
                                        op=mybir.AluOpType.mult)
                zn3 = nupool.tile([128, 128], F32, tag="zn3")
                nc.vector.tensor_tensor(zn3[:], zn2[:], cst["b_rep"][:],
                                        op=mybir.AluOpType.add)
                sg = nupool.tile([128, 128], F32, tag="sg")
                nc.scalar.activation(sg[:], zn3[:], AF.Sigmoid)
                nucsb = nupool.tile([128, 128], F32, tag="nucsb")
                nc.vector.tensor_tensor(nucsb[:], zn3[:], sg[:],
                                        op=mybir.AluOpType.mult)
                nrows = min(128, nuc_real - row0)
                if nrows > 0:
                    nc.sync.dma_start(o_nuc[row0:row0 + nrows, :],
                                      nucsb[0:nrows, :])
                ntp = ps_e.tile([128, 128], F32, tag="teps")
                nc.tensor.transpose(ntp[:], nucsb[:], cst["identf"][:])
                nc.vector.tensor_copy(nucT_sb[:, gt * 128:(gt + 1) * 128], ntp[:])

        # rot / tr heads
        for g in range(n_grp):
            c0 = g * grp_seg
            r1ps = ps_t.tile([128, grp_seg], F32, tag="tps")
            nc.tensor.matmul(r1ps[:], cst["rw1"][:],
                             nucT_sb[:, c0:c0 + grp_seg],
                             start=True, stop=True, skip_group_check=True)
            r1x = nupool.tile([128, grp_seg], F32, tag="r1x")
            nc.vector.tensor_scalar_add(r1x[:], r1ps[:], cst["rb1"][:, 0:1])
            r1g = nupool.tile([128, grp_seg], F32, tag="r1g")
            nc.scalar.activation(r1g[:], r1x[:], AF.Sigmoid)
            r1sb = nupool.tile([128, grp_seg], BF16, tag="r1sb")
            nc.vector.tensor_tensor(r1sb[:], r1x[:], r1g[:],
                                    op=mybir.AluOpType.mult)
            qtp = ps_s.tile([4, grp_seg], F32, tag="sps")
            nc.tensor.matmul(qtp[:], cst["rw2"][:], r1sb[:],
                             start=True, stop=True, skip_group_check=True)
            qsb = nupool.tile([4, grp_seg], F32, tag="qsb")
            nc.vector.tensor_scalar_add(qsb[:], qtp[:], cst["rb2"][:, 0:1])

            t1ps = ps_t.tile([128, grp_seg], F32, tag="tps")
            nc.tensor.matmul(t1ps[:], cst["tw1"][:],
                             nucT_sb[:, c0:c0 + grp_seg],
                             start=True, stop=True, skip_group_check=True)
            t1x = nupool.tile([128, grp_seg], F32, tag="r1x")
            nc.vector.tensor_scalar_add(t1x[:], t1ps[:], cst["tb1"][:, 0:1])
            t1g = nupool.tile([128, grp_seg], F32, tag="r1g")
            nc.scalar.activation(t1g[:], t1x[:], AF.Sigmoid)
            t1sb = nupool.tile([128, grp_seg], BF16, tag="r1sb")
            nc.vector.tensor_tensor(t1sb[:], t1x[:], t1g[:],
                                    op=mybir.AluOpType.mult)
            ttp = ps_s.tile([3, grp_seg], F32, tag="sps")
            nc.tensor.matmul(ttp[:], cst["tw2"][:], t1sb[:],
                             start=True, stop=True, skip_group_check=True)
            tsb = nupool.tile([3, grp_seg], F32, tag="tsb")
            nc.vector.tensor_scalar_add(tsb[:], ttp[:], cst["tb2"][:, 0:1])

            for t in range(grp_seg // 128):
                gt = g * (grp_seg // 128) + t
                row0 = gt * 128
                nrows = min(128, nuc_real - row0)
                if nrows <= 0:
                    continue
                qnp = ps_e.tile([128, 4], F32, tag="teps")
                nc.tensor.transpose(qnp[:], qsb[:, t * 128:(t + 1) * 128],
                                    cst["identf"][0:4, 0:4])
                qcp = nupool.tile([128, 4], F32, tag="qcp")
                nc.vector.tensor_copy(qcp[:], qnp[:])
                qsq = nupool.tile([128, 4], F32, tag="qsq")
                nc.vector.tensor_tensor(qsq[:], qcp[:], qcp[:],
                                        op=mybir.AluOpType.mult)
                ssq = nupool.tile([128, 1], F32, tag="ssq")
                nc.vector.reduce_sum(ssq[:], qsq[:], axis=AX.X)
                nrm = nupool.tile([128, 1], F32, tag="nrm")
                nc.scalar.activation(nrm[:], ssq[:], AF.Sqrt)
                nrmc = nupool.tile([128, 1], F32, tag="nrmc")
                nc.vector.tensor_scalar_max(nrmc[:], nrm[:], 1e-12)
                rn = nupool.tile([128, 1], F32, tag="rn")
                nc.vector.reciprocal(rn[:], nrmc[:])
                quat = nupool.tile([128, 4], F32, tag="quat")
                nc.vector.tensor_scalar_mul(quat[:], qcp[:], rn[:, 0:1])
                nc.sync.dma_start(o_quat[row0:row0 + nrows, :],
                                  quat[0:nrows, :])

                tnp = ps_e.tile([128, 4], F32, tag="teps")
                nc.tensor.transpose(tnp[:, 0:3],
                                    tsb[:, t * 128:(t + 1) * 128],
                                    cst["identf"][0:3, 0:3])
                trsb = nupool.tile([128, 3], F32, tag="trsb")
                nc.vector.tensor_copy(trsb[:], tnp[:, 0:3])
                nc.sync.dma_start(o_trans[row0:row0 + nrows, :],
                                  trsb[0:nrows, :])

    if not os.environ.get("KERNEL_NO_LEGALIZE"):
        _legalize_waits(nc)
    return nc


def _legalize_waits(nc, limit=1):
    """Walrus codegen allows only a couple of sem waits on engine ops.
    Move excess waits onto same-engine sequencer NOPs inserted just before."""
    Op = nc.isa.Opcode
    eng_map = {
        mybir.EngineType.DVE: nc.vector,
        mybir.EngineType.Activation: nc.scalar,
        mybir.EngineType.Pool: nc.gpsimd,
        mybir.EngineType.PE: nc.tensor,
        mybir.EngineType.SP: nc.sync,
    }
    for fn in nc.m.functions:
        for blk in fn.blocks:
            need = False
            for i in blk.instructions:
                si = getattr(i, "sync_info", None)
                if si and len(si.on_wait) > limit:
                    need = True
                    break
            if not need:
                continue
            out = []
            for inst in blk.instructions:
                si = getattr(inst, "sync_info", None)
                if (si and len(si.on_wait) > limit
                        and inst.engine in eng_map):
                    waits = list(si.on_wait)
                    keep, excess = waits[-limit:], waits[:-limit]
                    while excess:
                        chunk, excess = excess[:limit], excess[limit:]
                        bi = eng_map[inst.engine].isa(
                            Op.NEURON_ISA_TPB_OPCODE_NOP, {})
                        nop = bi.ins
                        for f2 in nc.m.functions:
                            for b2 in f2.blocks:
                                if nop in b2.instructions:
                                    b2.instructions.remove(nop)
                        nop.sync_info = mybir.SyncInfo(on_wait=chunk,
                                                       on_update=[])
                        out.append(nop)
                    inst.sync_info = mybir.SyncInfo(
                        on_wait=keep, on_update=list(si.on_update))
                out.append(inst)
            blk.instructions[:] = out


def _install_ntff_hook():
    """Recreate the missing antenv.axon_hooks module with a ctypes NTFF
    profile hook into libaxon_pjrt.so (mirrors trn_agent_boot.trn_boot)."""
    import types
    import ctypes
    import contextlib
    import antenv

    if "antenv.axon_hooks" in sys.modules:
        return
    so_path = "/opt/axon/libaxon_pjrt.so"
    mod = types.ModuleType("antenv.axon_hooks")
    state = {"hook": None}

    def set_axon_ntff_profile_hook(h):
        state["hook"] = h

    def get_axon_ntff_profile_hook():
        return state["hook"]

    mod.set_axon_ntff_profile_hook = set_axon_ntff_profile_hook
    mod.get_axon_ntff_profile_hook = get_axon_ntff_profile_hook
    sys.modules["antenv.axon_hooks"] = mod
    antenv.axon_hooks = mod

    try:
        lib = ctypes.CDLL(so_path)
        if not hasattr(lib, "axon_start_nrt_profile"):
            return
        lib.axon_start_nrt_profile.argtypes = [
            ctypes.POINTER(ctypes.c_int64), ctypes.c_size_t]
        lib.axon_start_nrt_profile.restype = ctypes.c_int64
        lib.axon_stop_nrt_profile.argtypes = [ctypes.c_char_p]
        lib.axon_stop_nrt_profile.restype = ctypes.c_int64

        @contextlib.contextmanager
        def _hook(output_dir, device_ids):
            import jax
            jax.devices()
            if device_ids:
                ids = (ctypes.c_int64 * len(device_ids))(*device_ids)
                rc = lib.axon_start_nrt_profile(ids, len(device_ids))
            else:
                rc = lib.axon_start_nrt_profile(None, 0)
            if rc != 0:
                raise RuntimeError(f"axon_start_nrt_profile rc={rc}")
            try:
                yield
            finally:
                n = lib.axon_stop_nrt_profile(str(output_dir).encode())
                print(f"ntff profile: {n} file(s) -> {output_dir}",
                      file=sys.stderr)

        state["hook"] = _hook
    except OSError:
        pass


# ---------------------------------------------------------------------------
# host side
# ---------------------------------------------------------------------------

def _pad2(a, r, c):
    out = np.zeros((r, c), dtype=np.float32)
    out[:a.shape[0], :a.shape[1]] = a
    return out


def make_weight_inputs(attn_W1, attn_b1, attn_W2, comb_W, comb_b,
                       sugar_W, sugar_b, phos_W, phos_b, ln_g, ln_b,
                       rot_W1, rot_b1, rot_W2, rot_b2,
                       tr_W1, tr_b1, tr_W2, tr_b2):
    f = lambda x: np.asarray(x, dtype=np.float32)
    attn_W1, attn_W2, comb_W = f(attn_W1), f(attn_W2), f(comb_W)
    c2 = comb_W[128:256]
    c3 = comb_W[256:384]
    ident = np.eye(128, dtype=np.float32)
    cst = {
        "W1e": attn_W1[10:138].astype(BF16_NP),
        "W1p": attn_W1[0:10].astype(BF16_NP),
        "W2": attn_W2.reshape(128, 1).astype(BF16_NP),
        "b1": f(attn_b1).reshape(128, 1),
        "S": _indicator_np(),
        "zrow": np.zeros((1, 512), dtype=BF16_NP),
        "identb": ident.astype(BF16_NP),
        "identf": ident,
        "C1": comb_W[0:128],
        "Wsug": f(sugar_W) @ c2,
        "Wpho": f(phos_W) @ c3,
        "bc": (f(comb_b) + f(sugar_b) @ c2 + f(phos_b) @ c3).reshape(128, 1),
        "g_rep": np.broadcast_to(f(ln_g), (128, 128)).copy(),
        "b_rep": np.broadcast_to(f(ln_b), (128, 128)).copy(),
        "eps_ln": np.full((128, 1), 1e-5, dtype=np.float32),
        "rw1": f(rot_W1).astype(BF16_NP),
        "rb1": f(rot_b1).reshape(128, 1),
        "rw2": f(rot_W2).astype(BF16_NP),
        "rb2": f(rot_b2).reshape(4, 1),
        "tw1": f(tr_W1).astype(BF16_NP),
        "tb1": f(tr_b1).reshape(128, 1),
        "tw2": f(tr_W2).astype(BF16_NP),
        "tb2": f(tr_b2).reshape(3, 1),
    }
    return cst


def make_core_inputs(physics_x, learnable_emb, sugar_x, phos_x,
                     core, nuc_real, n_st):
    nuc_pad = n_st * ST_SEG
    a_pad = nuc_pad * APN
    a_real = nuc_real * APN
    n0 = core * nuc_real
    a0 = n0 * APN
    emb = np.zeros((a_pad, 128), dtype=BF16_NP)
    emb[:a_real] = learnable_emb[a0:a0 + a_real].astype(BF16_NP)
    physT = np.zeros((10, a_pad), dtype=BF16_NP)
    physT[:, :a_real] = np.asarray(physics_x[a0:a0 + a_real],
                                   dtype=np.float32).T.astype(BF16_NP)
    sugxT = np.zeros((8, nuc_pad), dtype=np.float32)
    sugxT[:, :nuc_real] = np.asarray(sugar_x[n0:n0 + nuc_real],
                                     dtype=np.float32).T
    phoxT = np.zeros((8, nuc_pad), dtype=np.float32)
    phoxT[:, :nuc_real] = np.asarray(phos_x[n0:n0 + nuc_real],
                                     dtype=np.float32).T
    return {"emb": emb, "physT": physT, "sugxT": sugxT, "phoxT": phoxT}


_PROG_CACHE = {}
LAST_RESULTS = None


def kernel(physics_x, learnable_emb, atom_to_nuc, sugar_x, phos_x,
           num_nucleotides, sugar_W, sugar_b, phos_W, phos_b,
           attn_W1, attn_b1, attn_W2, attn_b2, comb_W, comb_b, ln_g, ln_b,
           rot_W1, rot_b1, rot_W2, rot_b2, tr_W1, tr_b1, tr_W2, tr_b2,
           _trace=False):
    global LAST_RESULTS
    physics_x = np.asarray(physics_x, dtype=np.float32)
    learnable_emb = np.asarray(learnable_emb, dtype=np.float32)
    sugar_x = np.asarray(sugar_x, dtype=np.float32)
    phos_x = np.asarray(phos_x, dtype=np.float32)

    nuc_real = NUC_C
    n_st = 20
    key = (n_st, nuc_real)
    if key not in _PROG_CACHE:
        _PROG_CACHE[key] = build_program(n_st, nuc_real)
    nc = _PROG_CACHE[key]

    wcst = make_weight_inputs(attn_W1, attn_b1, attn_W2, comb_W, comb_b,
                              sugar_W, sugar_b, phos_W, phos_b, ln_g, ln_b,
                              rot_W1, rot_b1, rot_W2, rot_b2,
                              tr_W1, tr_b1, tr_W2, tr_b2)
    in_maps = []
    for c in range(NCORES):
        m = dict(wcst)
        m.update(make_core_inputs(physics_x, learnable_emb, sugar_x, phos_x,
                                  c, nuc_real, n_st))
        in_maps.append(m)

    if _trace:
        _install_ntff_hook()
    res = run_bass_kernel_spmd(nc, in_maps, list(range(NCORES)), trace=_trace)
    LAST_RESULTS = res
    outs = res.results
    quat = np.concatenate([outs[c]["quat"] for c in range(NCORES)], axis=0)
    trans = np.concatenate([outs[c]["trans"] for c in range(NCORES)], axis=0)
    nuc = np.concatenate([outs[c]["nuc"] for c in range(NCORES)], axis=0)
    return quat, trans, nuc


# revision 26
# speedup vs baseline: 1.2102x; 1.2102x over previous
"""Trainium2 Bass kernel for AtomToNucleotideTransform (segment softmax-pool + MLPs).

Sharding: 8 cores, each owns a contiguous range of 2500 nucleotides (55000 atoms).
Per-core padded to 2560 segments / 56320 atoms (= 440 tiles of 128 atoms,
20 seg-tiles of 128 segments, supertile = 1408 atoms = 64 segs, lcm(22,128)).

Pipeline per core (all weights replicated):
  atom phase (per seg-tile = 2816 atoms = 22 atom-tiles):
    - DMA emb [128,128] f32 tiles; cast bf16 on GpSimd; PE-transpose -> embT
    - mm1: t = W1e.T @ embT (bf16) + W1p.T @ physT (f32r)  [H, atoms] psum
    - tanh (ACT, bias=b1) -> bf16
    - s: per-tile matmul lhsT=tanh-slice, rhs=W2 -> s_cols psum [128 atoms, 1]
    - segment softmax: s_cols -> (PE transpose + 2 coarse DMAs) -> [seg, 22]
      layout; max/exp/sum/recip on DVE+ACT; w back to per-atom columns
    - wemb = w * emb (DVE, bf16 out); pooled += wemb.T @ S_window (PE, psum
      accumulate over 512-seg groups)
  nuc phase: comb matmul (f32r, sugar/phos weights folded on host), LayerNorm
  (natural layout after PE transpose), SiLU, rot/tr heads, quaternion norm.
"""

import os
import sys
from contextlib import ExitStack

import numpy as np

sys.path.insert(0, "/opt/trn_rl_repo")

import ml_dtypes  # noqa: E402
import concourse.bass as bass  # noqa: E402
import concourse.tile as tile  # noqa: E402
from concourse import mybir  # noqa: E402
from concourse.bass_utils import run_bass_kernel_spmd  # noqa: E402

F32 = mybir.dt.float32
F32R = mybir.dt.float32r
BF16 = mybir.dt.bfloat16
AF = mybir.ActivationFunctionType
AX = mybir.AxisListType

H = 128
APN = 22
NCORES = 8
N_NUC = 20000
NUC_C = N_NUC // NCORES            # 2500 real nucleotides per core
SUP_SEG = 64                       # segs per supertile (lcm(22,128)/22)
SUP_TILES = 11                     # atom tiles per supertile
ST_SEG = 128                       # segs per seg-tile
ST_TILES = 22                      # atom tiles per seg-tile
GRP_SEG = 512                      # segs per psum accumulation group
BF16_NP = ml_dtypes.bfloat16

# supertile-periodic indicator pattern
_OFFS, _WIDS, _SCOL = [], [], [0]
for _j in range(SUP_TILES):
    _lo = (128 * _j) // APN
    _hi = (128 * _j + 127) // APN
    _OFFS.append(_lo)
    _WIDS.append(_hi - _lo + 1)
    _SCOL.append(_SCOL[-1] + _WIDS[-1])
SW = _SCOL[-1]


def _indicator_np():
    s = np.zeros((128, SW), dtype=np.float32)
    for j in range(SUP_TILES):
        for p in range(128):
            seg = (128 * j + p) // APN
            s[p, _SCOL[j] + seg - _OFFS[j]] = 1.0
    return s.astype(BF16_NP)


def build_program(n_st: int, nuc_real: int):
    """Emit the bass program for one core handling n_st seg-tiles
    (n_st*128 padded segments) with nuc_real real nucleotides."""
    nuc_pad = n_st * ST_SEG
    a_pad = nuc_pad * APN
    nt = a_pad // 128
    grp_seg = min(GRP_SEG, nuc_pad)
    stpg = grp_seg // ST_SEG          # seg-tiles per psum group
    n_grp = nuc_pad // grp_seg
    assert nuc_pad % grp_seg == 0 and nt == n_st * ST_TILES

    nc = bass.Bass()
    dram = {}

    def din(name, shape, dt=F32):
        dram[name] = nc.dram_tensor(name, list(shape), dt, kind="ExternalInput")
        return dram[name]

    din("emb", (a_pad, 128), BF16)
    din("physT", (10, a_pad), BF16)
    din("sugxT", (8, nuc_pad))
    din("phoxT", (8, nuc_pad))
    din("W1e", (128, 128), BF16)
    din("W1p", (10, 128), BF16)
    din("W2", (128, 1), BF16)
    din("b1", (128, 1))
    din("S", (128, SW), BF16)
    din("zrow", (1, 512), BF16)
    din("identb", (128, 128), BF16)
    din("identf", (128, 128))
    din("C1", (128, 128))
    din("Wsug", (8, 128))
    din("Wpho", (8, 128))
    din("bc", (128, 1))
    din("g_rep", (128, 128))
    din("b_rep", (128, 128))
    din("eps_ln", (128, 1))
    din("rw1", (128, 128), BF16)
    din("rb1", (128, 1))
    din("rw2", (128, 4), BF16)
    din("rb2", (4, 1))
    din("tw1", (128, 128), BF16)
    din("tb1", (128, 1))
    din("tw2", (128, 3), BF16)
    din("tb2", (3, 1))

    o_quat = nc.dram_tensor("quat", [nuc_real, 4], F32, kind="ExternalOutput")
    o_trans = nc.dram_tensor("trans", [nuc_real, 3], F32, kind="ExternalOutput")
    o_nuc = nc.dram_tensor("nuc", [nuc_real, 128], F32, kind="ExternalOutput")

    with tile.TileContext(nc) as tc, ExitStack() as ctx:
        cpool = ctx.enter_context(tc.tile_pool(name="consts", bufs=1))
        xpool = ctx.enter_context(tc.tile_pool(name="xf", bufs=3))
        etpool = ctx.enter_context(tc.tile_pool(name="embT", bufs=4))
        thpool = ctx.enter_context(tc.tile_pool(name="tanh", bufs=8))
        ptpool = ctx.enter_context(tc.tile_pool(name="physT", bufs=2))
        smpool = ctx.enter_context(tc.tile_pool(name="smax", bufs=5))
        wepool = ctx.enter_context(tc.tile_pool(name="wemb", bufs=4))
        nupool = ctx.enter_context(tc.tile_pool(name="nuc", bufs=4))
        bigpool = ctx.enter_context(tc.tile_pool(name="big", bufs=1))
        flpool = ctx.enter_context(tc.tile_pool(name="flat", bufs=3))
        ps_t = ctx.enter_context(tc.tile_pool(name="ps_t", bufs=2, space="PSUM"))
        ps_e = ctx.enter_context(tc.tile_pool(name="ps_e", bufs=2, space="PSUM"))
        ps_s = ctx.enter_context(tc.tile_pool(name="ps_s", bufs=2, space="PSUM"))
        ps_p = ctx.enter_context(tc.tile_pool(name="ps_p", bufs=2, space="PSUM"))

        # ---- load constants into SBUF
        cst = {}
        for name, dt in [
            ("W1e", BF16), ("W1p", BF16), ("W2", BF16), ("b1", F32),
            ("S", BF16), ("zrow", BF16), ("identb", BF16), ("identf", F32),
            ("C1", F32), ("Wsug", F32), ("Wpho", F32), ("bc", F32),
            ("g_rep", F32), ("b_rep", F32), ("eps_ln", F32),
            ("rw1", BF16), ("rb1", F32), ("rw2", BF16), ("rb2", F32),
            ("tw1", BF16), ("tb1", F32), ("tw2", BF16), ("tb2", F32),
        ]:
            t = cpool.tile(list(dram[name].shape), dt, tag=name)
            nc.sync.dma_start(t[:], dram[name][:])
            cst[name] = t

        pooled_sb = bigpool.tile([128, nuc_pad], F32, tag="pooled")
        nucT_sb = bigpool.tile([128, nuc_pad], BF16, tag="nucT")

        embv = dram["emb"]
        phv = dram["physT"]

        # batches of atom tiles within a seg-tile for the N-dim of mm1
        BATCHES = [(0, 4), (4, 4), (8, 4), (12, 4), (16, 4), (20, 2)]

        pooled_holder = [None]
        stash = {}

        # batches of atom tiles within a seg-tile for the N-dim of mm1
        def stage_a(st):
            """DMA + transpose + mm1 + tanh + attention logits for seg-tile st."""
            a0 = st * ST_TILES * 128
            xfst = xpool.tile([128, ST_TILES * 128], BF16, tag="xf")
            nc.sync.dma_start(
                xfst[:].rearrange("p (k f) -> p k f", f=128),
                embv[a0:a0 + ST_TILES * 128, :].rearrange(
                    "(k p) f -> p k f", p=128))
            ptst = ptpool.tile([10, ST_TILES * 128], BF16, tag="pt")
            nc.sync.dma_start(ptst[:], phv[:, a0:a0 + ST_TILES * 128])
            xf = [xfst[:, k * 128:(k + 1) * 128] for k in range(ST_TILES)]
            tanh_tiles = []
            for bi, (k0, knum) in enumerate(BATCHES):
                nb = knum * 128
                embT = etpool.tile([128, 512], BF16, tag="embT")
                nc.sync.dma_start(
                    embT[:, 0:nb],
                    embv[a0 + k0 * 128:a0 + k0 * 128 + nb, :],
                    transpose=True)
                tps = ps_t.tile([128, 512], F32, tag="tps")
                nc.tensor.matmul(tps[:, 0:nb], cst["W1e"][:], embT[:, 0:nb],
                                 start=True, stop=False, skip_group_check=True)
                nc.tensor.matmul(tps[:, 0:nb], cst["W1p"][:],
                                 ptst[:, k0 * 128:k0 * 128 + nb],
                                 start=False, stop=True, skip_group_check=True)
                th = thpool.tile([128, 512], BF16, tag="tanh")
                nc.scalar.activation(th[:, 0:nb], tps[:, 0:nb], AF.Tanh,
                                     bias=cst["b1"][:, 0:1])
                tanh_tiles.append((th, nb))
            scps = ps_s.tile([128, 160], F32, tag="sps")
            for k in range(ST_TILES):
                th, _ = tanh_tiles[k // 4]
                nc.tensor.matmul(
                    scps[:, k:k + 1], th[:, (k % 4) * 128:(k % 4 + 1) * 128],
                    cst["W2"][:], start=True, stop=True, skip_group_check=True)
            scols = smpool.tile([128, ST_TILES], F32, tag="scols")
            nc.vector.tensor_copy(scols[:], scps[:, 0:ST_TILES])
            stash[st] = (xf, scols)

        def stage_b(st):
            """Segment softmax + weighted pooling for seg-tile st."""
            xf, scols = stash.pop(st)
            if st % stpg == 0:
                pooled_holder[0] = ps_p.tile([128, grp_seg], F32,
                                             name="pooled_ps", tag="pooled_ps")
                nc.tensor.matmul(
                    pooled_holder[0][:, :], cst["zrow"][0:1, 0:128],
                    cst["zrow"][0:1, 0:grp_seg], start=True, stop=False,
                    skip_group_check=True)
            pooled_cur = pooled_holder[0]

            swps = ps_s.tile([128, 160], F32, tag="sps")
            nc.tensor.transpose(swps[0:ST_TILES, 0:128], scols[:],
                                cst["identf"][:])
            stsb = smpool.tile([ST_TILES, 128], F32, tag="stsb")
            nc.vector.tensor_copy(stsb[:], swps[0:ST_TILES, 0:128])
            sflat = flpool.tile([1, ST_TILES * 128], F32, tag="flat")
            nc.scalar.dma_start(
                sflat[0:1, :].rearrange("o (c p) -> o c p", p=128), stsb[:])
            sseg = smpool.tile([ST_SEG, APN], F32, tag="sseg")
            nc.scalar.dma_start(
                sseg[:], sflat[0:1, :].rearrange("o (n k) -> o n k", k=APN))

            mrow = smpool.tile([ST_SEG, 1], F32, tag="mrow")
            nc.vector.reduce_max(mrow[:], sseg[:], axis=AX.X)
            nm = smpool.tile([ST_SEG, 1], F32, tag="nm")
            nc.vector.tensor_scalar_mul(nm[:], mrow[:], -1.0)
            eseg = smpool.tile([ST_SEG, APN], F32, tag="eseg")
            den = smpool.tile([ST_SEG, 1], F32, tag="den")
            nc.scalar.activation(eseg[:], sseg[:], AF.Exp, bias=nm[:, 0:1],
                                 accum_out=den[:, 0:1])
            rden = smpool.tile([ST_SEG, 1], F32, tag="rden")
            nc.vector.reciprocal(rden[:], den[:])
            wseg = smpool.tile([ST_SEG, APN], F32, tag="wseg")
            nc.vector.tensor_scalar_mul(wseg[:], eseg[:], rden[:, 0:1])

            wflat = flpool.tile([1, ST_TILES * 128], F32, tag="flat")
            nc.scalar.dma_start(
                wflat[0:1, :].rearrange("o (n k) -> o n k", k=APN), wseg[:])
            wtsb = smpool.tile([ST_TILES, 128], F32, tag="wtsb")
            nc.scalar.dma_start(
                wtsb[:], wflat[0:1, :].rearrange("o (c p) -> o c p", p=128))
            nc.tensor.transpose(swps[:, 128:128 + ST_TILES], wtsb[:],
                                cst["identf"][0:ST_TILES, 0:ST_TILES])
            wcols = smpool.tile([128, ST_TILES], F32, tag="wcols")
            nc.vector.tensor_copy(wcols[:], swps[:, 128:128 + ST_TILES])

            for k in range(ST_TILES):
                jg = st * ST_TILES + k
                jj = jg % SUP_TILES
                seg0 = (jg // SUP_TILES) * SUP_SEG + _OFFS[jj]
                wdt = _WIDS[jj]
                col = seg0 - (st // stpg) * grp_seg
                we = wepool.tile([128, 128], BF16, tag="wemb")
                nc.vector.tensor_scalar_mul(we[:], xf[k], wcols[:, k:k + 1])
                nc.tensor.matmul(
                    pooled_cur[:, col:col + wdt], we[:],
                    cst["S"][:, _SCOL[jj]:_SCOL[jj] + wdt],
                    start=False,
                    stop=(st % stpg == stpg - 1 and k == ST_TILES - 1),
                    skip_group_check=True)

            if st % stpg == stpg - 1:
                g = st // stpg
                nc.vector.tensor_copy(
                    pooled_sb[:, g * grp_seg:(g + 1) * grp_seg],
                    pooled_cur[:])

        # ================= atom phase (software-pipelined) ================
        lag = 1
        for st in range(n_st):
            stage_a(st)
            if st >= lag:
                stage_b(st - lag)
        for st in range(n_st - lag, n_st):
            stage_b(st)

        # ================= nucleotide phase =================
        for g in range(n_grp):
            c0 = g * grp_seg
            sgx = ptpool.tile([8, grp_seg], F32, tag="sgx")
            nc.sync.dma_start(sgx[:], dram["sugxT"][:, c0:c0 + grp_seg])
            pgx = ptpool.tile([8, grp_seg], F32, tag="pgx")
            nc.sync.dma_start(pgx[:], dram["phoxT"][:, c0:c0 + grp_seg])
            zps = ps_t.tile([128, grp_seg], F32, tag="tps")
            nc.tensor.matmul(zps[:], cst["C1"][:],
                             pooled_sb[:, c0:c0 + grp_seg],
                             start=True, stop=False, skip_group_check=True)
            nc.tensor.matmul(zps[:], cst["Wsug"][:], sgx[:],
                             start=False, stop=False, skip_group_check=True)
            nc.tensor.matmul(zps[:], cst["Wpho"][:], pgx[:],
                             start=False, stop=True, skip_group_check=True)
            zsb = nupool.tile([128, grp_seg], F32, tag="zsb")
            nc.vector.tensor_scalar_add(zsb[:], zps[:], cst["bc"][:, 0:1])

            for t in range(grp_seg // 128):
                gt = g * stpg + t
                row0 = gt * 128
                ztp = ps_e.tile([128, 128], F32, tag="teps")
                nc.tensor.transpose(ztp[:], zsb[:, t * 128:(t + 1) * 128],
                                    cst["identf"][:])
                musum = nupool.tile([128, 1], F32, tag="musum")
                nc.vector.reduce_sum(musum[:], ztp[:], axis=AX.X)
                mu = nupool.tile([128, 1], F32, tag="mu")
                nc.vector.tensor_scalar_mul(mu[:], musum[:], 1.0 / 128.0)
                zc = nupool.tile([128, 128], F32, tag="zc")
                nc.vector.tensor_scalar_sub(zc[:], ztp[:], mu[:, 0:1])
                sq = nupool.tile([128, 128], F32, tag="sq")
                nc.vector.tensor_tensor(sq[:], zc[:], zc[:],
                                        op=mybir.AluOpType.mult)
                ssum = nupool.tile([128, 1], F32, tag="ssum")
                nc.vector.reduce_sum(ssum[:], sq[:], axis=AX.X)
                std = nupool.tile([128, 1], F32, tag="std")
                nc.scalar.activation(std[:], ssum[:], AF.Sqrt,
                                     bias=cst["eps_ln"][:, 0:1],
                                     scale=1.0 / 128.0)
                rstd = nupool.tile([128, 1], F32, tag="rstd")
                nc.vector.reciprocal(rstd[:], std[:])
                zn1 = nupool.tile([128, 128], F32, tag="zn1")
                nc.vector.tensor_scalar_mul(zn1[:], zc[:], rstd[:, 0:1])
                zn2 = nupool.tile([128, 128], F32, tag="zn2")
                nc.vector.tensor_tensor(zn2[:], zn1[:], cst["g_rep"][:],
                                        op=mybir.AluOpType.mult)
                zn3 = nupool.tile([128, 128], F32, tag="zn3")
                nc.vector.tensor_tensor(zn3[:], zn2[:], cst["b_rep"][:],
                                        op=mybir.AluOpType.add)
                sg = nupool.tile([128, 128], F32, tag="sg")
                nc.scalar.activation(sg[:], zn3[:], AF.Sigmoid)
                nucsb = nupool.tile([128, 128], F32, tag="nucsb")
                nc.vector.tensor_tensor(nucsb[:], zn3[:], sg[:],
                                        op=mybir.AluOpType.mult)
                nrows = min(128, nuc_real - row0)
                if nrows > 0:
                    nc.sync.dma_start(o_nuc[row0:row0 + nrows, :],
                                      nucsb[0:nrows, :])
                ntp = ps_e.tile([128, 128], F32, tag="teps")
                nc.tensor.transpose(ntp[:], nucsb[:], cst["identf"][:])
                nc.vector.tensor_copy(nucT_sb[:, gt * 128:(gt + 1) * 128], ntp[:])

        # rot / tr heads
        for g in range(n_grp):
            c0 = g * grp_seg
            r1ps = ps_t.tile([128, grp_seg], F32, tag="tps")
            nc.tensor.matmul(r1ps[:], cst["rw1"][:],
                             nucT_sb[:, c0:c0 + grp_seg],
                             start=True, stop=True, skip_group_check=True)
            r1x = nupool.tile([128, grp_seg], F32, tag="r1x")
            nc.vector.tensor_scalar_add(r1x[:], r1ps[:], cst["rb1"][:, 0:1])
            r1g = nupool.tile([128, grp_seg], F32, tag="r1g")
            nc.scalar.activation(r1g[:], r1x[:], AF.Sigmoid)
            r1sb = nupool.tile([128, grp_seg], BF16, tag="r1sb")
            nc.vector.tensor_tensor(r1sb[:], r1x[:], r1g[:],
                                    op=mybir.AluOpType.mult)
            qtp = ps_s.tile([4, grp_seg], F32, tag="sps")
            nc.tensor.matmul(qtp[:], cst["rw2"][:], r1sb[:],
                             start=True, stop=True, skip_group_check=True)
            qsb = nupool.tile([4, grp_seg], F32, tag="qsb")
            nc.vector.tensor_scalar_add(qsb[:], qtp[:], cst["rb2"][:, 0:1])

            t1ps = ps_t.tile([128, grp_seg], F32, tag="tps")
            nc.tensor.matmul(t1ps[:], cst["tw1"][:],
                             nucT_sb[:, c0:c0 + grp_seg],
                             start=True, stop=True, skip_group_check=True)
            t1x = nupool.tile([128, grp_seg], F32, tag="r1x")
            nc.vector.tensor_scalar_add(t1x[:], t1ps[:], cst["tb1"][:, 0:1])
            t1g = nupool.tile([128, grp_seg], F32, tag="r1g")
            nc.scalar.activation(t1g[:], t1x[:], AF.Sigmoid)
            t1sb = nupool.tile([128, grp_seg], BF16, tag="r1sb")
            nc.vector.tensor_tensor(t1sb[:], t1x[:], t1g[:],
                                    op=mybir.AluOpType.mult)
            ttp = ps_s.tile([3, grp_seg], F32, tag="sps")
            nc.tensor.matmul(ttp[:], cst["tw2"][:], t1sb[:],
                             start=True, stop=True, skip_group_check=True)
            tsb = nupool.tile([3, grp_seg], F32, tag="tsb")
            nc.vector.tensor_scalar_add(tsb[:], ttp[:], cst["tb2"][:, 0:1])

            for t in range(grp_seg // 128):
                gt = g * (grp_seg // 128) + t
                row0 = gt * 128
                nrows = min(128, nuc_real - row0)
                if nrows <= 0:
                    continue
                qnp = ps_e.tile([128, 4], F32, tag="teps")
                nc.tensor.transpose(qnp[:], qsb[:, t * 128:(t + 1) * 128],
                                    cst["identf"][0:4, 0:4])
                qcp = nupool.tile([128, 4], F32, tag="qcp")
                nc.vector.tensor_copy(qcp[:], qnp[:])
                qsq = nupool.tile([128, 4], F32, tag="qsq")
                nc.vector.tensor_tensor(qsq[:], qcp[:], qcp[:],
                                        op=mybir.AluOpType.mult)
                ssq = nupool.tile([128, 1], F32, tag="ssq")
                nc.vector.reduce_sum(ssq[:], qsq[:], axis=AX.X)
                nrm = nupool.tile([128, 1], F32, tag="nrm")
                nc.scalar.activation(nrm[:], ssq[:], AF.Sqrt)
                nrmc = nupool.tile([128, 1], F32, tag="nrmc")
                nc.vector.tensor_scalar_max(nrmc[:], nrm[:], 1e-12)
                rn = nupool.tile([128, 1], F32, tag="rn")
                nc.vector.reciprocal(rn[:], nrmc[:])
                quat = nupool.tile([128, 4], F32, tag="quat")
                nc.vector.tensor_scalar_mul(quat[:], qcp[:], rn[:, 0:1])
                nc.sync.dma_start(o_quat[row0:row0 + nrows, :],
                                  quat[0:nrows, :])

                tnp = ps_e.tile([128, 4], F32, tag="teps")
                nc.tensor.transpose(tnp[:, 0:3],
                                    tsb[:, t * 128:(t + 1) * 128],
                                    cst["identf"][0:3, 0:3])
                trsb = nupool.tile([128, 3], F32, tag="trsb")
                nc.vector.tensor_copy(trsb[:], tnp[:, 0:3])
                nc.sync.dma_start(o_trans[row0:row0 + nrows, :],
                                  trsb[0:nrows, :])

    if not os.environ.get("KERNEL_NO_LEGALIZE"):
        _legalize_waits(nc)
    return nc


def _legalize_waits(nc, limit=1):
    """Walrus codegen allows only a couple of sem waits on engine ops.
    Move excess waits onto same-engine sequencer NOPs inserted just before."""
    Op = nc.isa.Opcode
    eng_map = {
        mybir.EngineType.DVE: nc.vector,
        mybir.EngineType.Activation: nc.scalar,
        mybir.EngineType.Pool: nc.gpsimd,
        mybir.EngineType.PE: nc.tensor,
        mybir.EngineType.SP: nc.sync,
    }
    for fn in nc.m.functions:
        for blk in fn.blocks:
            need = False
            for i in blk.instructions:
                si = getattr(i, "sync_info", None)
                if si and len(si.on_wait) > limit:
                    need = True
                    break
            if not need:
                continue
            out = []
            for inst in blk.instructions:
                si = getattr(inst, "sync_info", None)
                if (si and len(si.on_wait) > limit
                        and inst.engine in eng_map):
                    waits = list(si.on_wait)
                    keep, excess = waits[-limit:], waits[:-limit]
                    while excess:
                        chunk, excess = excess[:limit], excess[limit:]
                        bi = eng_map[inst.engine].isa(
                            Op.NEURON_ISA_TPB_OPCODE_NOP, {})
                        nop = bi.ins
                        for f2 in nc.m.functions:
                            for b2 in f2.blocks:
                                if nop in b2.instructions:
                                    b2.instructions.remove(nop)
                        nop.sync_info = mybir.SyncInfo(on_wait=chunk,
                                                       on_update=[])
                        out.append(nop)
                    inst.sync_info = mybir.SyncInfo(
                        on_wait=keep, on_update=list(si.on_update))
                out.append(inst)
            blk.instructions[:] = out


def _install_ntff_hook():
    """Recreate the missing antenv.axon_hooks module with a ctypes NTFF
    profile hook into libaxon_pjrt.so (mirrors trn_agent_boot.trn_boot)."""
    import types
    import ctypes
    import contextlib
    import antenv

    if "antenv.axon_hooks" in sys.modules:
        return
    so_path = "/opt/axon/libaxon_pjrt.so"
    mod = types.ModuleType("antenv.axon_hooks")
    state = {"hook": None}

    def set_axon_ntff_profile_hook(h):
        state["hook"] = h

    def get_axon_ntff_profile_hook():
        return state["hook"]

    mod.set_axon_ntff_profile_hook = set_axon_ntff_profile_hook
    mod.get_axon_ntff_profile_hook = get_axon_ntff_profile_hook
    sys.modules["antenv.axon_hooks"] = mod
    antenv.axon_hooks = mod

    try:
        lib = ctypes.CDLL(so_path)
        if not hasattr(lib, "axon_start_nrt_profile"):
            return
        lib.axon_start_nrt_profile.argtypes = [
            ctypes.POINTER(ctypes.c_int64), ctypes.c_size_t]
        lib.axon_start_nrt_profile.restype = ctypes.c_int64
        lib.axon_stop_nrt_profile.argtypes = [ctypes.c_char_p]
        lib.axon_stop_nrt_profile.restype = ctypes.c_int64

        @contextlib.contextmanager
        def _hook(output_dir, device_ids):
            import jax
            jax.devices()
            if device_ids:
                ids = (ctypes.c_int64 * len(device_ids))(*device_ids)
                rc = lib.axon_start_nrt_profile(ids, len(device_ids))
            else:
                rc = lib.axon_start_nrt_profile(None, 0)
            if rc != 0:
                raise RuntimeError(f"axon_start_nrt_profile rc={rc}")
            try:
                yield
            finally:
                n = lib.axon_stop_nrt_profile(str(output_dir).encode())
                print(f"ntff profile: {n} file(s) -> {output_dir}",
                      file=sys.stderr)

        state["hook"] = _hook
    except OSError:
        pass


# ---------------------------------------------------------------------------
# host side
# ---------------------------------------------------------------------------

def _pad2(a, r, c):
    out = np.zeros((r, c), dtype=np.float32)
    out[:a.shape[0], :a.shape[1]] = a
    return out


def make_weight_inputs(attn_W1, attn_b1, attn_W2, comb_W, comb_b,
                       sugar_W, sugar_b, phos_W, phos_b, ln_g, ln_b,
                       rot_W1, rot_b1, rot_W2, rot_b2,
                       tr_W1, tr_b1, tr_W2, tr_b2):
    f = lambda x: np.asarray(x, dtype=np.float32)
    attn_W1, attn_W2, comb_W = f(attn_W1), f(attn_W2), f(comb_W)
    c2 = comb_W[128:256]
    c3 = comb_W[256:384]
    ident = np.eye(128, dtype=np.float32)
    cst = {
        "W1e": attn_W1[10:138].astype(BF16_NP),
        "W1p": attn_W1[0:10].astype(BF16_NP),
        "W2": attn_W2.reshape(128, 1).astype(BF16_NP),
        "b1": f(attn_b1).reshape(128, 1),
        "S": _indicator_np(),
        "zrow": np.zeros((1, 512), dtype=BF16_NP),
        "identb": ident.astype(BF16_NP),
        "identf": ident,
        "C1": comb_W[0:128],
        "Wsug": f(sugar_W) @ c2,
        "Wpho": f(phos_W) @ c3,
        "bc": (f(comb_b) + f(sugar_b) @ c2 + f(phos_b) @ c3).reshape(128, 1),
        "g_rep": np.broadcast_to(f(ln_g), (128, 128)).copy(),
        "b_rep": np.broadcast_to(f(ln_b), (128, 128)).copy(),
        "eps_ln": np.full((128, 1), 1e-5, dtype=np.float32),
        "rw1": f(rot_W1).astype(BF16_NP),
        "rb1": f(rot_b1).reshape(128, 1),
        "rw2": f(rot_W2).astype(BF16_NP),
        "rb2": f(rot_b2).reshape(4, 1),
        "tw1": f(tr_W1).astype(BF16_NP),
        "tb1": f(tr_b1).reshape(128, 1),
        "tw2": f(tr_W2).astype(BF16_NP),
        "tb2": f(tr_b2).reshape(3, 1),
    }
    return cst


def make_core_inputs(physics_x, learnable_emb, sugar_x, phos_x,
                     core, nuc_real, n_st):
    nuc_pad = n_st * ST_SEG
    a_pad = nuc_pad * APN
    a_real = nuc_real * APN
    n0 = core * nuc_real
    a0 = n0 * APN
    emb = np.zeros((a_pad, 128), dtype=BF16_NP)
    emb[:a_real] = learnable_emb[a0:a0 + a_real].astype(BF16_NP)
    physT = np.zeros((10, a_pad), dtype=BF16_NP)
    physT[:, :a_real] = np.asarray(physics_x[a0:a0 + a_real],
                                   dtype=np.float32).T.astype(BF16_NP)
    sugxT = np.zeros((8, nuc_pad), dtype=np.float32)
    sugxT[:, :nuc_real] = np.asarray(sugar_x[n0:n0 + nuc_real],
                                     dtype=np.float32).T
    phoxT = np.zeros((8, nuc_pad), dtype=np.float32)
    phoxT[:, :nuc_real] = np.asarray(phos_x[n0:n0 + nuc_real],
                                     dtype=np.float32).T
    return {"emb": emb, "physT": physT, "sugxT": sugxT, "phoxT": phoxT}


_PROG_CACHE = {}
LAST_RESULTS = None


def kernel(physics_x, learnable_emb, atom_to_nuc, sugar_x, phos_x,
           num_nucleotides, sugar_W, sugar_b, phos_W, phos_b,
           attn_W1, attn_b1, attn_W2, attn_b2, comb_W, comb_b, ln_g, ln_b,
           rot_W1, rot_b1, rot_W2, rot_b2, tr_W1, tr_b1, tr_W2, tr_b2,
           _trace=False):
    global LAST_RESULTS
    physics_x = np.asarray(physics_x, dtype=np.float32)
    learnable_emb = np.asarray(learnable_emb, dtype=np.float32)
    sugar_x = np.asarray(sugar_x, dtype=np.float32)
    phos_x = np.asarray(phos_x, dtype=np.float32)

    nuc_real = NUC_C
    n_st = 20
    key = (n_st, nuc_real)
    if key not in _PROG_CACHE:
        _PROG_CACHE[key] = build_program(n_st, nuc_real)
    nc = _PROG_CACHE[key]

    wcst = make_weight_inputs(attn_W1, attn_b1, attn_W2, comb_W, comb_b,
                              sugar_W, sugar_b, phos_W, phos_b, ln_g, ln_b,
                              rot_W1, rot_b1, rot_W2, rot_b2,
                              tr_W1, tr_b1, tr_W2, tr_b2)
    in_maps = []
    for c in range(NCORES):
        m = dict(wcst)
        m.update(make_core_inputs(physics_x, learnable_emb, sugar_x, phos_x,
                                  c, nuc_real, n_st))
        in_maps.append(m)

    if _trace:
        _install_ntff_hook()
    res = run_bass_kernel_spmd(nc, in_maps, list(range(NCORES)), trace=_trace)
    LAST_RESULTS = res
    outs = res.results
    quat = np.concatenate([outs[c]["quat"] for c in range(NCORES)], axis=0)
    trans = np.concatenate([outs[c]["trans"] for c in range(NCORES)], axis=0)
    nuc = np.concatenate([outs[c]["nuc"] for c in range(NCORES)], axis=0)
    return quat, trans, nuc
